# revision 1
# baseline (speedup 1.0000x reference)
"""GatedSparseAttention Trainium2 kernel (8-core SPMD, Bass/Tile).

Sharding: sequence-parallel over queries with stride-8 interleave so every
core's causal structure is identical (SPMD-uniform). Core c owns rows
{c+8z} U {1024+c+8z} for z in [0,128). K/V/Ki are computed on own rows and
all-gathered (k-axis lands in permuted order p=256*cc+z <-> g, which is
chunk-causal: even 128-chunks hold g<1024, odd chunks g>=1024).

Numerics: indexer path fp32 (selection exactness), attention path bf16.
Top-k via 22-iteration per-row bisection with fused compare+count
(DVE tensor_scalar accum / ACT Sign accum), then masked softmax via
PSUM bias matmul (-240*mask, scale 1/8 in ACT exp) and PV with [v|1]
augmented lhsT giving the softmax denominator for free.
"""
import math
import os
import sys

sys.path.insert(0, "/opt/trn_rl_repo")
import numpy as np
import ml_dtypes

import concourse.bass as bass
import concourse.mybir as mybir
from concourse import bacc
from concourse.tile import TileContext
from concourse.bass_utils import run_bass_kernel_spmd

F32 = mybir.dt.float32
F32R = mybir.dt.float32r
BF16 = mybir.dt.bfloat16
AX = mybir.AluOpType
AF = mybir.ActivationFunctionType

T, D, H, DH = 2048, 512, 8, 64
NI, DI, KSEL = 4, 64, 128
NC = 8
Z = 256          # own rows per core
NB = 20          # bisection iterations
INV = 1.0 / math.sqrt(DI)
SCALE = DH ** -0.5
MASK_BIG = 240.0  # -240 * (1/8 ACT scale) = -30 on masked logits

W0, W1 = 1024, 2048  # score widths for q-tile 0 / 1


def rows_for_core(c):
    z = np.arange(Z)
    return (z // 128) * 1024 + c + 8 * (z % 128)


def kcb(t, b):
    """k column-block b of q-tile t -> (cc, z0) into gathered [8,256] layout."""
    if t == 0:
        return b, 0
    return b // 2, 128 * (b % 2)


def build(debug=False, loop=1, sim=False, nb=None, skip_att=False):
    nc = bacc.Bacc("TRN2", target_bir_lowering=False, debug=False, num_devices=NC)

    # ---------------- DRAM parameters ----------------
    P = {}

    def par(name, shape, dt):
        P[name] = nc.declare_dram_parameter(name, list(shape), dt, isOutput=False)
        return P[name]

    par("xtf", (512, Z), F32)        # x.T own cols (perm order), fp32
    par("xtb", (512, Z), BF16)       # bf16 copy
    par("wq", (512, 512), BF16)      # RoPE-even part: Wq (plain)
    par("wq2", (512, 512), BF16)     # Wq @ S (rot-half folded)
    par("wk", (512, 512), BF16)
    par("wk2", (512, 512), BF16)
    par("wv", (512, 512), BF16)
    par("wvg", (512, 512), BF16)
    par("wog", (512, 512), BF16)
    par("wo", (512, 512), BF16)
    par("wiq", (512, 256), F32)
    par("wik", (512, 64), F32)
    par("wiw", (512, 4), F32)
    par("bvg_row", (1, 512), BF16)
    par("biw_row", (1, 4), F32)
    par("bogt", (64, 8), F32)        # bog per-head per-partition form
    par("idxb_r", (4, Z), F32)       # idx_bias[h]/INV replicated rows
    par("cosq2", (128, Z), BF16)     # cos for own rows, tiled x2 heads
    par("sinq2", (128, Z), BF16)
    par("vmask0", (128, W0), F32)    # causal validity q-tile0 {0,1}
    par("vmask1", (128, W1), F32)
    par("ident", (128, 128), BF16)   # identity
    par("identB", (128, 128), BF16)  # MASK_BIG * identity
    par("onesb", (1, 128), BF16)
    par("onesf", (1, 128), F32)

    out_t = nc.declare_dram_parameter("outT", [512, Z], F32, isOutput=True)
    dbg = {}
    if debug:
        for nm, sh, dt in [
            ("d_scores0", (128, W0), F32), ("d_scores1", (128, W1), F32),
            ("d_lo", (128, 2), F32), ("d_cnt", (128, 2), F32),
            ("d_kt", (512, 2048), F32), ("d_ki", (64, 2048), F32),
            ("d_vg", (256, 512), F32), ("d_qt", (512, 256), F32),
            ("d_og", (512, 256), F32), ("d_gated", (512, 256), F32),
            ("d_wsig", (256, 4), F32),
        ]:
            dbg[nm] = nc.declare_dram_parameter(nm, list(sh), dt, isOutput=True)

    # internal DRAM for collectives
    g_k_in = nc.dram_tensor("g_k_in", [512, Z], BF16)
    g_k_out = nc.dram_tensor("g_k_out", [NC, 512, Z], BF16, addr_space="Shared")
    g_v_in = nc.dram_tensor("g_v_in", [Z, 512], BF16)
    g_v_out = nc.dram_tensor("g_v_out", [NC, Z, 512], BF16, addr_space="Shared")
    g_ki_in = nc.dram_tensor("g_ki_in", [64, Z], F32)
    g_ki_out = nc.dram_tensor("g_ki_out", [NC, 64, Z], F32, addr_space="Shared")

    with TileContext(nc) as tc:
        with (
            tc.tile_pool(name="wpool", bufs=1) as wpool,      # persistent weights
            tc.tile_pool(name="big", bufs=1) as big,          # persistent activations
            tc.tile_pool(name="work", bufs=3) as work,        # transient sbuf
            tc.tile_pool(name="small", bufs=4) as small,      # tiny tiles
            tc.tile_pool(name="ps", bufs=2, space="PSUM") as ps,
            tc.tile_pool(name="ps_big", bufs=2, space="PSUM") as ps_big,
            tc.tile_pool(name="ps_pv", bufs=2, space="PSUM") as ps_pv,
        ):
            # ---------- load weights / constants ----------
            def load(name, shape, dt, src):
                t_ = wpool.tile(shape, dt, tag=name, name=name)
                nc.sync.dma_start(out=t_[:], in_=src)
                return t_

            xtf = [load(f"xtf{i}", [128, Z], F32, P["xtf"][128 * i:128 * (i + 1), :]) for i in range(4)]
            xtb = [load(f"xtb{i}", [128, Z], BF16, P["xtb"][128 * i:128 * (i + 1), :]) for i in range(4)]
            wsb = {}
            for w in ["wq", "wq2", "wk", "wk2", "wv", "wvg", "wog"]:
                wsb[w] = [load(f"{w}_{i}", [128, 512], BF16, P[w][128 * i:128 * (i + 1), :]) for i in range(4)]
            wo_h = [load(f"wo_{i}", [64, 512], BF16, P["wo"][64 * i:64 * (i + 1), :]) for i in range(8)]
            wiq = [load(f"wiq{i}", [128, 256], F32, P["wiq"][128 * i:128 * (i + 1), :]) for i in range(4)]
            wik = [load(f"wik{i}", [128, 64], F32, P["wik"][128 * i:128 * (i + 1), :]) for i in range(4)]
            wiw = [load(f"wiw{i}", [128, 4], F32, P["wiw"][128 * i:128 * (i + 1), :]) for i in range(4)]
            bvg_row = load("bvg_row", [1, 512], BF16, P["bvg_row"][:])
            biw_row = load("biw_row", [1, 4], F32, P["biw_row"][:])
            bogt = load("bogt", [64, 8], F32, P["bogt"][:])
            ones65 = wpool.tile([65, 128], F32, tag="ones65", name="ones65")
            nc.vector.memset(ones65[:], 1.0)
            cosq2 = load("cosq2", [128, Z], BF16, P["cosq2"][:])
            sinq2 = load("sinq2", [128, Z], BF16, P["sinq2"][:])
            vmask = [load("vmask0", [128, W0], F32, P["vmask0"][:]),
                     load("vmask1", [128, W1], F32, P["vmask1"][:])]
            ident = load("ident", [128, 128], BF16, P["ident"][:])
            identB = load("identB", [128, 128], BF16, P["identB"][:])
            onesb = load("onesb", [1, 128], BF16, P["onesb"][:])
            onesf = load("onesf", [1, 128], F32, P["onesf"][:])

            # ---------- own projections ----------
            # qT / kT (d-major, RoPE via W and W@S), -> bf16 [4][128, Z]
            def dmajor_rope(wname, w2name, tag):
                outs = []
                for dc in range(4):
                    p1 = ps.tile([128, Z], F32, tag="ps")
                    for dn in range(4):
                        nc.tensor.matmul(out=p1[:], lhsT=wsb[wname][dn][:, 128 * dc:128 * (dc + 1)],
                                         rhs=xtb[dn][:], start=(dn == 0), stop=(dn == 3))
                    p2 = ps.tile([128, Z], F32, tag="ps")
                    for dn in range(4):
                        nc.tensor.matmul(out=p2[:], lhsT=wsb[w2name][dn][:, 128 * dc:128 * (dc + 1)],
                                         rhs=xtb[dn][:], start=(dn == 0), stop=(dn == 3))
                    a = work.tile([128, Z], F32, tag="rope_a", bufs=2)
                    nc.vector.tensor_tensor(out=a[:], in0=p1[:], in1=cosq2[:], op=AX.mult)
                    b = work.tile([128, Z], F32, tag="rope_b", bufs=2)
                    nc.vector.tensor_tensor(out=b[:], in0=p2[:], in1=sinq2[:], op=AX.mult)
                    o = big.tile([128, Z], BF16, tag=f"{tag}{dc}", name=f"{tag}{dc}")
                    nc.vector.tensor_tensor(out=o[:], in0=a[:], in1=b[:], op=AX.add)
                    outs.append(o)
                return outs

            qt = dmajor_rope("wq", "wq2", "qt")
            kt = dmajor_rope("wk", "wk2", "kt")
            for dc in range(4):
                nc.sync.dma_start(out=g_k_in[128 * dc:128 * (dc + 1), :], in_=kt[dc][:])

            # v gated (row-major) -> bounce
            for zc in range(2):
                pv_ = ps.tile([128, 512], F32, tag="ps")
                for dn in range(4):
                    nc.tensor.matmul(out=pv_[:], lhsT=xtb[dn][:, 128 * zc:128 * (zc + 1)],
                                     rhs=wsb["wv"][dn][:], start=(dn == 0), stop=(dn == 3))
                pg_ = ps.tile([128, 512], F32, tag="ps")
                for dn in range(4):
                    nc.tensor.matmul(out=pg_[:], lhsT=xtb[dn][:, 128 * zc:128 * (zc + 1)],
                                     rhs=wsb["wvg"][dn][:], start=(dn == 0), stop=False)
                nc.tensor.matmul(out=pg_[:], lhsT=onesb[:], rhs=bvg_row[:], start=False, stop=True)
                sg = work.tile([128, 512], BF16, tag="vg_sig", bufs=2)
                nc.scalar.activation(out=sg[:], in_=pg_[:], func=AF.Sigmoid)
                vg = work.tile([128, 512], BF16, tag="vg_out", bufs=2)
                nc.vector.tensor_tensor(out=vg[:], in0=pv_[:], in1=sg[:], op=AX.mult)
                nc.sync.dma_start(out=g_v_in[128 * zc:128 * (zc + 1), :], in_=vg[:])
                if debug:
                    vgf = work.tile([128, 512], F32, tag="vg_dbg", bufs=1)
                    nc.vector.tensor_copy(out=vgf[:], in_=vg[:])
                    nc.sync.dma_start(out=dbg["d_vg"][128 * zc:128 * (zc + 1), :], in_=vgf[:])

            # kiT own (fp32) -> bounce
            pki = ps.tile([64, Z], F32, tag="ps")
            for dn in range(4):
                nc.tensor.matmul(out=pki[:], lhsT=wik[dn][:],
                                 rhs=xtf[dn][:], start=(dn == 0), stop=(dn == 3))
            ki_own = work.tile([64, Z], F32, tag="ki_own", bufs=1)
            nc.scalar.copy(out=ki_own[:], in_=pki[:])
            nc.sync.dma_start(out=g_ki_in[:], in_=ki_own[:])

            # qi per-head [4][64, Z] fp32
            qih = []
            for h in range(NI):
                pq = ps.tile([64, Z], F32, tag="ps")
                for dn in range(4):
                    nc.tensor.matmul(out=pq[:], lhsT=wiq[dn][:, 64 * h:64 * (h + 1)],
                                     rhs=xtf[dn][:], start=(dn == 0), stop=(dn == 3))
                qq = big.tile([64, Z], F32, tag=f"qih{h}", name=f"qih{h}")
                nc.scalar.copy(out=qq[:], in_=pq[:])
                qih.append(qq)

            # w_sig q-major [2][128, 4] fp32 + bisect init
            w_sig, hi_t, lo_t = [], None, None
            lohi = big.tile([128, 4], F32, tag="lohi")  # cols: lo0 lo1 hi0 hi1
            wrow2 = big.tile([128, 2], F32, tag="wrow2")
            for t in range(2):
                pw = ps.tile([128, 4], F32, tag="ps")
                for dn in range(4):
                    nc.tensor.matmul(out=pw[:], lhsT=xtf[dn][:, 128 * t:128 * (t + 1)],
                                     rhs=wiw[dn][:], start=(dn == 0), stop=False)
                nc.tensor.matmul(out=pw[:], lhsT=onesf[:], rhs=biw_row[:], start=False, stop=True)
                ws = big.tile([128, 4], F32, tag=f"wsig{t}", name=f"wsig{t}")
                nc.scalar.activation(out=ws[:], in_=pw[:], func=AF.Sigmoid)
                w_sig.append(ws)
                nc.vector.reduce_sum(out=wrow2[:, t:t + 1], in_=ws[:], axis=mybir.AxisListType.X)
                if debug:
                    nc.sync.dma_start(out=dbg["d_wsig"][128 * t:128 * (t + 1), :], in_=ws[:])

            # ---------- collectives ----------
            if sim:
                for cc in range(NC):
                    nc.sync.dma_start(out=g_ki_out[cc], in_=g_ki_in[:])
                    nc.sync.dma_start(out=g_k_out[cc], in_=g_k_in[:])
                    nc.sync.dma_start(out=g_v_out[cc], in_=g_v_in[:])
            else:
                nc.gpsimd.collective_compute("AllGather", AX.bypass,
                                             replica_groups=[list(range(NC))],
                                             ins=[g_ki_in[:]], outs=[g_ki_out[:]])
                nc.gpsimd.collective_compute("AllGather", AX.bypass,
                                             replica_groups=[list(range(NC))],
                                             ins=[g_k_in[:]], outs=[g_k_out[:]])
                nc.gpsimd.collective_compute("AllGather", AX.bypass,
                                             replica_groups=[list(range(NC))],
                                             ins=[g_v_in[:]], outs=[g_v_out[:]])

            # gathered kiT -> [64, 8, 256] fp32
            kiT = big.tile([64, NC, Z], F32, tag="kiT")
            nc.sync.dma_start(out=kiT[:, :, :], in_=g_ki_out[:, :, :].rearrange("c d z -> d c z"))

            # gathered kT -> [4][128, 8, 256] bf16
            ktf = []
            for dc in range(4):
                kk = big.tile([128, NC, Z], BF16, tag=f"ktf{dc}", name=f"ktf{dc}")
                nc.sync.dma_start(out=kk[:, :, :],
                                  in_=g_k_out[:, 128 * dc:128 * (dc + 1), :].rearrange("c d z -> d c z"))
                ktf.append(kk)

            # gathered v -> [16][128, 8, 65] bf16 ([v|1] per head)
            vaug = []
            for j in range(16):
                vv = big.tile([128, H, 65], BF16, tag=f"vaug{j}", name=f"vaug{j}")
                cc, zh = j // 2, j % 2
                nc.sync.dma_start(
                    out=vv[:, :, 0:64],
                    in_=g_v_out[cc, 128 * zh:128 * zh + 128, :].rearrange("z (h d) -> z h d", h=H))
                nc.vector.memset(vv[:, :, 64:65], 1.0)
                vaug.append(vv)

            if debug:
                for dc in range(4):
                    tmp = work.tile([128, 2048], F32, tag="dbg_kt", bufs=1)
                    nc.vector.tensor_copy(out=tmp[:], in_=ktf[dc][:].rearrange("d c z -> d (c z)"))
                    nc.sync.dma_start(out=dbg["d_kt"][128 * dc:128 * (dc + 1), :], in_=tmp[:])
                tmpk = work.tile([64, 2048], F32, tag="dbg_ki", bufs=1)
                nc.vector.tensor_copy(out=tmpk[:], in_=kiT[0:64].rearrange("d c z -> d (c z)"))
                nc.sync.dma_start(out=dbg["d_ki"][:], in_=tmpk[:])
                for dc in range(4):
                    tq = work.tile([128, 256], F32, tag="dbg_qt", bufs=1)
                    nc.vector.tensor_copy(out=tq[:], in_=qt[dc][:])
                    nc.sync.dma_start(out=dbg["d_qt"][128 * dc:128 * (dc + 1), :], in_=tq[:])

            import contextlib
            loop_cm = (tc.For_i(0, loop, 1)
                       if loop > 1 else contextlib.nullcontext())
            with loop_cm:
                # per-iteration bisect bracket init
                nc.vector.tensor_scalar(out=lohi[:, 2:4], in0=wrow2[:], scalar1=1.001,
                                        scalar2=1e-4, op0=AX.mult, op1=AX.add)
                ogh = []  # og per-head bf16 [8][64, Z]
                for h in range(H):
                    po = ps.tile([64, Z], F32, tag="ps")
                    for dn in range(4):
                        nc.tensor.matmul(out=po[:], lhsT=wsb["wog"][dn][:, 64 * h:64 * (h + 1)],
                                         rhs=xtb[dn][:], start=(dn == 0), stop=(dn == 3))
                    oo = big.tile([64, Z], BF16, tag=f"og{h}", name=f"og{h}")
                    nc.scalar.activation(out=oo[:], in_=po[:], func=AF.Sigmoid, bias=bogt[:, h:h + 1])
                    ogh.append(oo)
                    if debug:
                        of = work.tile([64, Z], F32, tag="og_dbg", bufs=1)
                        nc.vector.tensor_copy(out=of[:], in_=oo[:])
                        nc.sync.dma_start(out=dbg["d_og"][64 * h:64 * (h + 1), :], in_=of[:])
                nc.vector.memset(lohi[:, 0:2], 0.0)
                # ---------- indexer scores (q-major, fp32) ----------
                scores = [big.tile([128, W0], F32, tag="scores0", name="scores0"),
                          big.tile([128, W1], F32, tag="scores1", name="scores1")]
                for t in range(2):
                    nrb = (W0 if t == 0 else W1) // 256
                    for rb in range(nrb):
                        if t == 0:
                            rhs = kiT[:, 2 * rb:2 * rb + 2, 0:128]
                        else:
                            rhs = kiT[:, rb:rb + 1, :]
                        acc = scores[t][:, 256 * rb:256 * (rb + 1)]
                        for hf in range(2):
                            praw = ps_big.tile([128, 2, 256], F32, tag="psb")
                            for hh in range(2):
                                h = 2 * hf + hh
                                nc.tensor.matmul(out=praw[:, hh, :],
                                                 lhsT=qih[h][:, 128 * t:128 * (t + 1)],
                                                 rhs=rhs,
                                                 start=True, stop=True)
                            g2 = work.tile([128, 2, 256], F32, tag="gated2", bufs=2)
                            nc.scalar.activation(out=g2[:], in_=praw[:], func=AF.Sigmoid, scale=INV)
                            for hh in range(2):
                                h = 2 * hf + hh
                                if h == 0:
                                    nc.vector.tensor_scalar(out=acc, in0=g2[:, 0, :], scalar1=w_sig[t][:, 0:1],
                                                            scalar2=None, op0=AX.mult)
                                else:
                                    nc.vector.scalar_tensor_tensor(out=acc, in0=g2[:, hh, :],
                                                                   scalar=w_sig[t][:, h:h + 1],
                                                                   in1=acc, op0=AX.mult, op1=AX.add)
                    # causal validity (zero out invalid)
                    nc.gpsimd.tensor_tensor(out=scores[t][:], in0=scores[t][:], in1=vmask[t][:], op=AX.mult)

                # ---------- bisection (22 iters) ----------
                W_t = [W0, W1]
                for it in range(NB if nb is None else nb):
                    mid = small.tile([128, 2], F32, tag="mid")
                    nc.vector.tensor_tensor(out=mid[:], in0=lohi[:, 0:2], in1=lohi[:, 2:4], op=AX.add)
                    nc.vector.tensor_scalar(out=mid[:], in0=mid[:], scalar1=0.5, scalar2=None, op0=AX.mult)
                    nmid = small.tile([128, 2], F32, tag="nmid")
                    nc.vector.tensor_scalar(out=nmid[:], in0=mid[:], scalar1=-1.0, scalar2=None, op0=AX.mult)
                    cnt = small.tile([128, 2], F32, tag="cnt")
                    # tile0 on DVE: fused compare+count (STT accum)
                    d0 = work.tile([128, W0], BF16, tag="bis_d0", bufs=1)
                    nc.vector.scalar_tensor_tensor(out=d0[:], in0=scores[0][:], scalar=mid[:, 0:1],
                                                   in1=scores[0][:], op0=AX.is_gt,
                                                   op1=AX.logical_and, accum_out=cnt[:, 0:1])
                    # tile1 on ACT: Sign + accum; cnt_gt = (acc + W1)/2
                    d1 = work.tile([128, W1], BF16, tag="bis_d1", bufs=1)
                    nc.scalar.activation(out=d1[:], in_=scores[1][:], func=AF.Sign,
                                         bias=nmid[:, 1:2], accum_out=cnt[:, 1:2])
                    nc.vector.tensor_scalar(out=cnt[:, 1:2], in0=cnt[:, 1:2], scalar1=float(W1),
                                            scalar2=0.5, op0=AX.add, op1=AX.mult)
                    pred = small.tile([128, 2], F32, tag="pred")
                    nc.vector.tensor_scalar(out=pred[:], in0=cnt[:], scalar1=float(KSEL),
                                            scalar2=None, op0=AX.is_ge)
                    pm = small.tile([128, 2], F32, tag="pm")
                    nc.vector.scalar_tensor_tensor(out=pm[:], in0=pred[:], scalar=1.0,
                                                   in1=mid[:], op0=AX.bypass, op1=AX.mult)
                    nc.vector.tensor_tensor(out=lohi[:, 0:2], in0=lohi[:, 0:2], in1=pm[:], op=AX.max)
                    pm2 = small.tile([128, 2], F32, tag="pm2")
                    nc.vector.scalar_tensor_tensor(out=pm2[:], in0=pred[:], scalar=1e9,
                                                   in1=mid[:], op0=AX.mult, op1=AX.add)
                    nc.vector.tensor_tensor(out=lohi[:, 2:4], in0=lohi[:, 2:4], in1=pm2[:], op=AX.min)
                    if debug and it == NB - 1:
                        nc.sync.dma_start(out=dbg["d_cnt"][:], in_=cnt[:])

                if debug:
                    nc.sync.dma_start(out=dbg["d_scores0"][:], in_=scores[0][:])
                    nc.sync.dma_start(out=dbg["d_scores1"][:], in_=scores[1][:])
                    nc.sync.dma_start(out=dbg["d_lo"][:], in_=lohi[:, 0:2])

                # ---------- selection mask bias {0,-1} + transpose ----------
                bias_q = [work.tile([128, W0], BF16, tag="biasq0", name="biasq0", bufs=1),
                          work.tile([128, W1], BF16, tag="biasq1", name="biasq1", bufs=1)]
                for t in range(2):
                    nc.vector.tensor_scalar(out=bias_q[t][:], in0=scores[t][:], scalar1=lohi[:, t:t + 1],
                                            scalar2=-1.0, op0=AX.is_le, op1=AX.mult)
                biasTj = {}
                for j in range(16):
                    cc, zh = j // 2, j % 2
                    if zh == 0:
                        # even: tile0 col block cc gives rows (q of tile0), tile1 block 2*cc
                        pt = ps.tile([128, 256], BF16, tag="ps", name="pt")
                        nc.tensor.transpose(out=pt[:, 0:128], in_=bias_q[0][:, 128 * cc:128 * (cc + 1)],
                                            identity=ident[:])
                        nc.tensor.transpose(out=pt[:, 128:256], in_=bias_q[1][:, 256 * cc:256 * cc + 128],
                                            identity=ident[:])
                        bt = big.tile([128, 256], BF16, tag=f"biasTj{j}", name=f"biasTj{j}")
                        nc.vector.tensor_copy(out=bt[:], in_=pt[:])
                    else:
                        pt = ps.tile([128, 256], BF16, tag="ps", name="pt")
                        nc.tensor.transpose(out=pt[:, 0:128],
                                            in_=bias_q[1][:, 256 * cc + 128:256 * (cc + 1)],
                                            identity=ident[:])
                        bt = big.tile([128, 128], BF16, tag=f"biasTj{j}", name=f"biasTj{j}")
                        nc.vector.tensor_copy(out=bt[:], in_=pt[:, 0:128])
                    biasTj[j] = bt

                # ---------- attention ----------
                # joint q-tiles: even chunks serve both tiles (256 q cols),
                # odd chunks serve tile1 only (cols 128:256)
                gated = {}
                for h in range(H):
                    dc, hh = h // 2, h % 2
                    pv_ps = ps_pv.tile([65, 256], F32, tag="pv")
                    qrhs = qt[dc][64 * hh:64 * hh + 64, :]          # [64, 256]
                    qrhs1 = qt[dc][64 * hh:64 * hh + 64, 128:256]   # [64, 128] tile1
                    # even chunks (cc, zh=0): joint
                    for g0 in range(0, 8, 2):
                        pl = ps.tile([128, 512], F32, tag="ps")
                        for bi in range(2):
                            cc = g0 + bi
                            sl = pl[:, 256 * bi:256 * (bi + 1)]
                            nc.tensor.matmul(out=sl, lhsT=ktf[dc][64 * hh:64 * hh + 64, cc, 0:128],
                                             rhs=qrhs, start=True, stop=False)
                            nc.tensor.matmul(out=sl, lhsT=identB[:],
                                             rhs=biasTj[2 * cc][:], start=False, stop=True)
                        e = work.tile([128, 512], BF16, tag="e_tile", bufs=4)
                        nc.scalar.activation(out=e[:], in_=pl[:], func=AF.Exp, scale=SCALE)
                        for bi in range(2):
                            cc = g0 + bi
                            nc.tensor.matmul(out=pv_ps[:], lhsT=vaug[2 * cc][:, h, :],
                                             rhs=e[:, 256 * bi:256 * (bi + 1)],
                                             start=(cc == 0), stop=False)
                    # odd chunks (cc, zh=1): tile1 only
                    for g0 in range(0, 8, 4):
                        pl = ps.tile([128, 512], F32, tag="ps")
                        for bi in range(4):
                            cc = g0 + bi
                            sl = pl[:, 128 * bi:128 * (bi + 1)]
                            nc.tensor.matmul(out=sl, lhsT=ktf[dc][64 * hh:64 * hh + 64, cc, 128:256],
                                             rhs=qrhs1, start=True, stop=False)
                            nc.tensor.matmul(out=sl, lhsT=identB[:],
                                             rhs=biasTj[2 * cc + 1][:], start=False, stop=True)
                        e = work.tile([128, 512], BF16, tag="e_tile", bufs=4)
                        nc.scalar.activation(out=e[:], in_=pl[:], func=AF.Exp, scale=SCALE)
                        for bi in range(4):
                            cc = g0 + bi
                            nc.tensor.matmul(out=pv_ps[:, 128:256], lhsT=vaug[2 * cc + 1][:, h, :],
                                             rhs=e[:, 128 * bi:128 * (bi + 1)],
                                             start=False, stop=(g0 + bi == 7))
                    # normalize + og (partition-aligned)
                    zs = work.tile([65, 256], F32, tag="zs", bufs=2)
                    nc.vector.reciprocal(out=zs[64:65, :], in_=pv_ps[64:65, :])
                    pz = ps_pv.tile([64, 256], F32, tag="pvz", bufs=1)
                    nc.tensor.matmul(out=pz[:], lhsT=ones65[64:65, 0:64],
                                     rhs=zs[64:65, :], start=True, stop=True)
                    ozr = work.tile([64, 256], F32, tag="ozr", bufs=2)
                    nc.vector.tensor_tensor(out=ozr[:], in0=ogh[h][:], in1=pz[:], op=AX.mult)
                    g_ht = big.tile([64, 256], BF16, tag=f"gated{h}", name=f"gated{h}")
                    nc.vector.scalar_tensor_tensor(out=g_ht[:],
                                                   in0=pv_ps[0:64, :], scalar=1.0, in1=ozr[:],
                                                   op0=AX.bypass, op1=AX.mult)
                    gated[h] = g_ht

                # ---------- output projection ----------
                if skip_att:
                    for dc in range(4):
                        zf = work.tile([128, Z], F32, tag="out_f0", bufs=1)
                        nc.vector.tensor_scalar(out=zf[:], in0=bias_q[1][:, 0:Z], scalar1=1.0,
                                                scalar2=None, op0=AX.mult)
                        nc.sync.dma_start(out=out_t[128 * dc:128 * (dc + 1), :], in_=zf[:])
                for dc in (range(4) if not skip_att else range(0)):
                    po = ps.tile([128, Z], F32, tag="ps")
                    for h in range(H):
                        nc.tensor.matmul(out=po[:],
                                         lhsT=wo_h[h][:, 128 * dc:128 * (dc + 1)],
                                         rhs=gated[h][:], start=(h == 0), stop=(h == H - 1))
                    of = work.tile([128, Z], F32, tag="out_f", bufs=2)
                    nc.vector.tensor_copy(out=of[:], in_=po[:])
                    nc.sync.dma_start(out=out_t[128 * dc:128 * (dc + 1), :], in_=of[:])

    nc.compile()
    return nc


# ======================= host side =======================

def _bf(a):
    return np.asarray(a, ml_dtypes.bfloat16)


def host_inputs(x, Wq, Wk, Wv, Wo, Wiq, Wik, Wiw, biw, idx_bias, Wvg, bvg, Wog, bog):
    """Build per-core in_maps. x: [T, D] fp32."""
    Tl, Dl = x.shape
    xT = np.ascontiguousarray(x.T)

    # rot-half fold matrix S (block-diag per head): (k @ S) = rot_half(k)
    S1 = np.zeros((DH, DH), np.float32)
    for d in range(32):
        S1[d + 32, d] = -1.0
    for d in range(32, 64):
        S1[d - 32, d] = 1.0
    S = np.kron(np.eye(H, dtype=np.float32), S1)

    inv_freq = 1.0 / (10000.0 ** (np.arange(0, DH, 2, dtype=np.float32) / DH))
    t_ar = np.arange(Tl, dtype=np.float32)
    fr = np.outer(t_ar, inv_freq)
    emb = np.concatenate([fr, fr], -1)
    cos_t, sin_t = np.cos(emb).astype(np.float32), np.sin(emb).astype(np.float32)

    com = {
        "wq": _bf(Wq), "wq2": _bf(Wq @ S), "wk": _bf(Wk), "wk2": _bf(Wk @ S),
        "wv": _bf(Wv), "wvg": _bf(Wvg), "wog": _bf(Wog), "wo": _bf(Wo),
        "wiq": np.ascontiguousarray(Wiq, np.float32),
        "wik": np.ascontiguousarray(Wik, np.float32),
        "wiw": np.ascontiguousarray(Wiw, np.float32),
        "bvg_row": _bf(bvg[None, :]),
        "biw_row": np.ascontiguousarray(biw[None, :], np.float32),
        "bogt": np.ascontiguousarray(bog.reshape(8, 64).T, np.float32),
        "ident": _bf(np.eye(128, dtype=np.float32)),
        "identB": _bf(MASK_BIG * np.eye(128, dtype=np.float32)),
        "onesb": _bf(np.ones((1, 128), np.float32)),
        "onesf": np.ones((1, 128), np.float32),
        "idxb_r": np.ascontiguousarray(
            np.repeat((idx_bias / INV).astype(np.float32)[:, None], Z, axis=1)),
    }

    in_maps = []
    for c in range(NC):
        rows = rows_for_core(c)
        m = dict(com)
        m["xtf"] = np.ascontiguousarray(xT[:, rows], np.float32)
        m["xtb"] = _bf(m["xtf"])
        cos2 = np.tile(cos_t[rows].T, (2, 1))      # [128, 256]
        sin2 = np.tile(sin_t[rows].T, (2, 1))
        m["cosq2"] = _bf(cos2)
        m["sinq2"] = _bf(sin2)
        # validity masks
        gq0 = rows[:128]                            # [128]
        gq1 = rows[128:]
        # tile0 columns: m -> cc = m//128, z = m%128, g = cc + 8z
        cc0 = np.arange(W0) // 128
        zz0 = np.arange(W0) % 128
        gk0 = cc0 + 8 * zz0
        m["vmask0"] = (gk0[None, :] <= gq0[:, None]).astype(np.float32)
        # tile1 columns: p -> cc = p//256, z = p%256, g = (z//128)*1024 + cc + 8*(z%128)
        pp = np.arange(W1)
        cc1 = pp // 256
        z1 = pp % 256
        gk1 = (z1 // 128) * 1024 + cc1 + 8 * (z1 % 128)
        m["vmask1"] = (gk1[None, :] <= gq1[:, None]).astype(np.float32)
        in_maps.append(m)
    return in_maps


def assemble(results):
    out = np.zeros((T, D), np.float32)
    for c in range(NC):
        rows = rows_for_core(c)
        out[rows, :] = results[c]["outT"].T
    return out


# ======================= harness entry =======================

_CACHE = {}


def _get_nc(loop=1):
    if loop not in _CACHE:
        _CACHE[loop] = build(debug=False, loop=loop)
    return _CACHE[loop]


def _run(in_maps, loop=1):
    nc = _get_nc(loop)
    return run_bass_kernel_spmd(nc, in_maps, list(range(NC)))


def kernel(x, Wq, Wk, Wv, Wo, Wiq, Wik, Wiw, biw, idx_bias, Wvg, bvg, Wog, bog):
    """Full-input entry: shards across 8 NeuronCores internally."""
    x = np.asarray(x, np.float32)
    B, Tl, Dl = x.shape
    in_maps = host_inputs(
        x[0], np.asarray(Wq, np.float32), np.asarray(Wk, np.float32),
        np.asarray(Wv, np.float32), np.asarray(Wo, np.float32),
        np.asarray(Wiq, np.float32), np.asarray(Wik, np.float32),
        np.asarray(Wiw, np.float32), np.asarray(biw, np.float32),
        np.asarray(idx_bias, np.float32), np.asarray(Wvg, np.float32),
        np.asarray(bvg, np.float32), np.asarray(Wog, np.float32),
        np.asarray(bog, np.float32))
    res = _run(in_maps, loop=1)
    return assemble(res.results).reshape(B, Tl, Dl)

